# revision 7
# baseline (speedup 1.0000x reference)
"""Trainium2 Bass kernel for nn_ContrastiveCosineLoss.

loss = mean_{i<j} (cos(f_i,f_j) - cos(r_i,r_j))^2 over N=2048 rows.

Math: with Fn/Rn the row-normalized embeddings and
  Gf = Fn^T Fn  [1024,1024],  Gr = Rn^T Rn  [128,128],  X = Fn^T Rn  [1024,128]
the pairwise-difference matrix D = Fn Fn^T - Rn Rn^T satisfies
  ||D||_F^2 = ||Gf||_F^2 + ||Gr||_F^2 - 2||X||_F^2
and loss = (||D||_F^2 - sum_i D_ii^2) / (2M), M = N(N-1)/2. The diagonal term
is (|fn_i|^2-|rn_i|^2)^2 ~ 1e-14 against ||D||^2 ~ 4e4 and is dropped.

This avoids the [N,N] matrix entirely: only feature-space Grams are computed,
consuming the row-major inputs directly (matmul takes lhsT, so the natural
layout IS the transposed-lhs layout). Row normalization folds into the
stationary operand only:
  Gf slice  = (a.Fc)^T F   a = 1/max(nf,eps)^2   (Fc = 128 cols of F)
  X  sliceT = (g.Rc)^T F   g = 1/(max(nf,eps) max(nr,eps))
  Gr slice  = (b.Rc)^T R   b = 1/max(nr,eps)^2
All scales are per-row (per contraction index), so they may sit on either
matmul operand; putting them on the narrow slice keeps elementwise work low
and the moving operand is the raw streamed chunk (N=512 -> fp32r fast path).

Sharding (8 cores, SPMD single program, no collectives): every core streams
the full F[2048,1024] + R[2048,128] (9MB, needed for the row norms anyway)
and owns Gf rows c*128.., X cols c*16.., Gr rows c*16... Per-core column
windows are realized WITHOUT dynamic addressing by giving each core a
column-rotated copy of F and R (np.roll, axis=1) -- Frobenius norms are
invariant under the induced permutations. Each core emits 5 partial sums
(Gf lo/hi, X lo/hi, Gr); the host combines them.
"""

import numpy as np

N_ROWS = 2048
KF = 1024
KR = 128
P = 128
NCH = N_ROWS // P          # 16 contraction chunks
GF_W = 128                 # Gf rows per core (lhsT cols)
X_W = 16                   # X / Gr rows per core
M_PAIRS = N_ROWS * (N_ROWS - 1) // 2
EPS2 = 1e-16               # max(norm,1e-8)^2 clamp, applied to norm^2
GRP = 2                    # chunks per norm/scale batch
ACT_SQ = {0, 1, 4, 5, 8, 9, 12, 13, 14, 15}  # F-square chunks on ScalarE

TRACE = False              # test.py flips this (needs the axon NTFF shim)
LAST_EXEC_NS = None

_CACHED_NC = None


def _build():
    import concourse.bacc as bacc
    import concourse.mybir as mybir
    from concourse.tile import TileContext
    from concourse.alu_op_type import AluOpType

    F32 = mybir.dt.float32
    F32R = mybir.dt.float32r
    ACTF = mybir.ActivationFunctionType
    AX = mybir.AxisListType

    nc = bacc.Bacc("TRN2", num_devices=8)
    fa = nc.dram_tensor("fa", [N_ROWS, KF], F32, kind="ExternalInput")
    ra = nc.dram_tensor("ra", [N_ROWS, KR], F32, kind="ExternalInput")
    out = nc.dram_tensor("out", [5, 1], F32, kind="ExternalOutput")

    with TileContext(nc) as tc:
        with (
            tc.tile_pool(name="fa_p", bufs=16) as fa_p,
            tc.tile_pool(name="big_p", bufs=1) as big_p,
            tc.tile_pool(name="lhs_p", bufs=6) as lhs_p,
            tc.tile_pool(name="nrm_p", bufs=4) as nrm_p,
            tc.tile_pool(name="scl_p", bufs=4) as scl_p,
            tc.tile_pool(name="scr_p", bufs=4) as scr_p,
            tc.tile_pool(name="acc_p", bufs=1) as acc_p,
            tc.tile_pool(name="psum", bufs=6, space="PSUM") as psum_p,
        ):
            # --- constants / accumulators ---
            acc5 = acc_p.tile([P, 5], F32)
            ones = acc_p.tile([P, 1], F32)
            nc.vector.memset(ones[:], 1.0)
            nc.vector.memset(acc5[:], 0.0)
            # prime the ACT table set (sqrt_and_others: Square+Sqrt+Copy)
            # so the ~2.7us table load overlaps the first DMAs.
            warm = acc_p.tile([P, 1], F32)
            nc.scalar.activation(warm[:], ones[:], ACTF.Sqrt)

            # --- R, all chunks in one [128, 16*128] tile; batch norms ---
            ra_all = big_p.tile([P, NCH * KR], F32R)
            nc.gpsimd.dma_start(
                ra_all[:].rearrange("p (k j) -> p k j", j=KR),
                ra[:].rearrange("(k p) j -> p k j", p=P).bitcast(F32R),
            )
            rsq = big_p.tile([P, NCH * KR], F32)
            nc.vector.tensor_tensor(
                rsq[:], ra_all[:].bitcast(F32), ra_all[:].bitcast(F32),
                AluOpType.mult,
            )
            nr2 = nrm_p.tile([P, NCH], F32, tag="nr2")
            nc.vector.reduce_sum(
                nr2[:], rsq[:].rearrange("p (k j) -> p k j", j=KR), axis=AX.X
            )
            tr_all = nrm_p.tile([P, NCH], F32, tag="tr")
            br_all = nrm_p.tile([P, NCH], F32, tag="br")
            nc.vector.tensor_scalar_max(tr_all[:], nr2[:], EPS2)
            nc.vector.reciprocal(br_all[:], tr_all[:])

            # --- PSUM accumulators (5 banks) ---
            psA0 = psum_p.tile([P, 512], F32, tag="acc", name="psA0")
            psA1 = psum_p.tile([P, 512], F32, tag="acc", name="psA1")
            psX0 = psum_p.tile([P, 512], F32, tag="acc", name="psX0")
            psX1 = psum_p.tile([P, 512], F32, tag="acc", name="psX1")
            psB = psum_p.tile([P, KR], F32, tag="acc", name="psB")

            # --- issue every F chunk DMA up-front; transfers pipeline ---
            fa_sbs = [None] * NCH
            for ki in range(NCH):
                fa_sb = fa_p.tile([P, KF], F32R, tag="fa_sb", name=f"fa{ki}")
                nc.sync.dma_start(
                    fa_sb[:], fa[ki * P:(ki + 1) * P, :].bitcast(F32R)
                )
                fa_sbs[ki] = fa_sb

            # --- main stream: per group of GRP chunks ---
            for g in range(NCH // GRP):
                nf2 = nrm_p.tile([P, GRP], F32, tag="nf2")
                for j in range(GRP):
                    ki = GRP * g + j
                    fa_sb = fa_sbs[ki]
                    if ki in ACT_SQ:
                        scr = scr_p.tile([P, KF], F32, tag="scrA")
                        nc.scalar.activation(
                            scr[:], fa_sb[:].bitcast(F32), ACTF.Square,
                            accum_out=nf2[:, j:j + 1],
                        )
                    else:
                        scr = scr_p.tile([P, KF], F32, tag="scrV")
                        nc.vector.scalar_tensor_tensor(
                            scr[:], fa_sb[:].bitcast(F32), 1.0,
                            fa_sb[:].bitcast(F32),
                            AluOpType.mult, AluOpType.mult,
                            accum_out=nf2[:, j:j + 1],
                        )

                # batched scale math for this group
                tf = scl_p.tile([P, GRP], F32, tag="tf")
                af = scl_p.tile([P, GRP], F32, tag="af")
                uu = scl_p.tile([P, GRP], F32, tag="uu")
                ss = scl_p.tile([P, GRP], F32, tag="ss")
                gg = scl_p.tile([P, GRP], F32, tag="gg")
                nc.vector.tensor_scalar_max(tf[:], nf2[:], EPS2)
                nc.vector.reciprocal(af[:], tf[:])
                nc.vector.tensor_tensor(
                    uu[:], tf[:], tr_all[:, GRP * g:GRP * (g + 1)],
                    AluOpType.mult,
                )
                nc.scalar.activation(ss[:], uu[:], ACTF.Sqrt)
                nc.vector.reciprocal(gg[:], ss[:])

                # scaled stationary slices + the 5 matmuls per chunk
                for j in range(GRP):
                    ki = GRP * g + j
                    fa_sb = fa_sbs[ki]
                    la = lhs_p.tile([P, GF_W], F32R, tag="la")
                    nc.vector.tensor_scalar_mul(
                        la[:], fa_sb[:, 0:GF_W].bitcast(F32), af[:, j:j + 1]
                    )
                    lx = lhs_p.tile([P, X_W], F32R, tag="lx")
                    nc.vector.tensor_scalar_mul(
                        lx[:],
                        ra_all[:, ki * KR:ki * KR + X_W].bitcast(F32),
                        gg[:, j:j + 1],
                    )
                    lb = lhs_p.tile([P, X_W], F32R, tag="lb")
                    nc.vector.tensor_scalar_mul(
                        lb[:],
                        ra_all[:, ki * KR:ki * KR + X_W].bitcast(F32),
                        br_all[:, ki:ki + 1],
                    )
                    st = dict(start=(ki == 0), stop=(ki == NCH - 1))
                    nc.tensor.matmul(psA0[:], lhsT=la[:], rhs=fa_sb[:, 0:512], **st)
                    nc.tensor.matmul(psA1[:], lhsT=la[:], rhs=fa_sb[:, 512:KF], **st)
                    nc.tensor.matmul(psX0[0:X_W, :], lhsT=lx[:], rhs=fa_sb[:, 0:512], **st)
                    nc.tensor.matmul(psX1[0:X_W, :], lhsT=lx[:], rhs=fa_sb[:, 512:KF], **st)
                    nc.tensor.matmul(
                        psB[0:X_W, :], lhsT=lb[:],
                        rhs=ra_all[:, ki * KR:(ki + 1) * KR], **st
                    )

            # --- epilogue: Frobenius partials into acc5 cols ---
            for col, (ps, rows, w) in enumerate([
                (psA0, P, 512), (psA1, P, 512),
                (psX0, X_W, 512), (psX1, X_W, 512), (psB, X_W, KR),
            ]):
                scr = scr_p.tile([P, w], F32, tag="scrE", name=f"scrE{col}")
                nc.scalar.activation(
                    scr[0:rows, :], ps[0:rows, 0:w], ACTF.Square,
                    accum_out=acc5[0:rows, col:col + 1],
                )

            # partition-reduce via ones-matmul: out[5,1] = acc5^T @ ones
            psum_s = psum_p.tile([5, 1], F32, tag="acc", name="psS")
            nc.tensor.matmul(
                psum_s[:], lhsT=acc5[:], rhs=ones[:], start=True, stop=True
            )
            out_sb = acc_p.tile([5, 1], F32)
            nc.scalar.copy(out_sb[:], psum_s[:])
            nc.sync.dma_start(out[:], out_sb[:])

    nc.finalize()
    return nc


def kernel(reduced_embeddings: np.ndarray, full_embeddings: np.ndarray) -> np.ndarray:
    global _CACHED_NC, LAST_EXEC_NS
    from concourse.bass_utils import run_bass_kernel_spmd

    F = np.ascontiguousarray(full_embeddings, dtype=np.float32)
    R = np.ascontiguousarray(reduced_embeddings, dtype=np.float32)

    if _CACHED_NC is None:
        _CACHED_NC = _build()
    nc = _CACHED_NC

    # Shard: core c sees F rotated left by c*128 cols, R rotated by c*16.
    in_maps = []
    for c in range(8):
        fa = np.roll(F, -(c * GF_W), axis=1)
        ra = np.roll(R, -(c * X_W), axis=1)
        in_maps.append({"fa": np.ascontiguousarray(fa), "ra": np.ascontiguousarray(ra)})

    kw = {}
    if TRACE:
        kw = dict(trace=True, trace_cores=[0])
    res = run_bass_kernel_spmd(nc, in_maps, core_ids=list(range(8)), **kw)
    LAST_EXEC_NS = res.exec_time_ns

    # out rows: [Gf_lo, Gf_hi, X_lo, X_hi, Gr]; every core's piece is distinct.
    s_gf = sum(float(res.results[c]["out"][0, 0] + res.results[c]["out"][1, 0]) for c in range(8))
    s_x = sum(float(res.results[c]["out"][2, 0] + res.results[c]["out"][3, 0]) for c in range(8))
    s_gr = sum(float(res.results[c]["out"][4, 0]) for c in range(8))
    loss = (s_gf - 2.0 * s_x + s_gr) / (2.0 * M_PAIRS)
    return np.float32(loss)


# revision 8
# speedup vs baseline: 1.1574x; 1.1574x over previous
"""Trainium2 Bass kernel for nn_ContrastiveCosineLoss.

loss = mean_{i<j} (cos(f_i,f_j) - cos(r_i,r_j))^2 over N=2048 rows.

Math: with Fn/Rn the row-normalized embeddings and
  Gf = Fn^T Fn  [1024,1024],  Gr = Rn^T Rn  [128,128],  X = Fn^T Rn  [1024,128]
the pairwise-difference matrix D = Fn Fn^T - Rn Rn^T satisfies
  ||D||_F^2 = ||Gf||_F^2 + ||Gr||_F^2 - 2||X||_F^2
and loss = (||D||_F^2 - sum_i D_ii^2) / (2M), M = N(N-1)/2. The diagonal term
is (|fn_i|^2-|rn_i|^2)^2 ~ 1e-14 against ||D||^2 ~ 4e4 and is dropped.

This avoids the [N,N] matrix entirely: only feature-space Grams are computed,
consuming the row-major inputs directly (matmul takes lhsT, so the natural
layout IS the transposed-lhs layout). Row normalization folds into the
stationary operand only:
  Gf slice  = (a.Fc)^T F   a = 1/max(nf,eps)^2   (Fc = 128 cols of F)
  X  sliceT = (g.Rc)^T F   g = 1/(max(nf,eps) max(nr,eps))
  Gr slice  = (b.Rc)^T R   b = 1/max(nr,eps)^2
All scales are per-row (per contraction index), so they may sit on either
matmul operand; putting them on the narrow slice keeps elementwise work low
and the moving operand is the raw streamed chunk (N=512 -> fp32r fast path).

Sharding (8 cores, SPMD single program, no collectives): every core streams
the full F[2048,1024] + R[2048,128] (9MB, needed for the row norms anyway)
and owns Gf rows c*128.., X cols c*16.., Gr rows c*16... Per-core column
windows are realized WITHOUT dynamic addressing by giving each core a
column-rotated copy of F and R (np.roll, axis=1) -- Frobenius norms are
invariant under the induced permutations. Each core emits 5 partial sums
(Gf lo/hi, X lo/hi, Gr); the host combines them.
"""

import numpy as np

N_ROWS = 2048
KF = 1024
KR = 128
P = 128
NCH = N_ROWS // P          # 16 contraction chunks
GF_W = 128                 # Gf rows per core (lhsT cols)
X_W = 16                   # X / Gr rows per core
M_PAIRS = N_ROWS * (N_ROWS - 1) // 2
EPS2 = 1e-16               # max(norm,1e-8)^2 clamp, applied to norm^2
GRP = 2                    # chunks per norm/scale batch
ACT_SQ = {0, 1, 4, 5, 8, 9, 12, 13, 14, 15}  # F-square chunks on ScalarE

TRACE = False              # test.py flips this (needs the axon NTFF shim)
LAST_EXEC_NS = None

_CACHED_NC = None


def _build():
    import concourse.bacc as bacc
    import concourse.mybir as mybir
    from concourse.tile import TileContext
    from concourse.alu_op_type import AluOpType

    F32 = mybir.dt.float32
    F32R = mybir.dt.float32r
    ACTF = mybir.ActivationFunctionType
    AX = mybir.AxisListType

    nc = bacc.Bacc("TRN2", num_devices=8)
    fa = nc.dram_tensor("fa", [N_ROWS, KF], F32, kind="ExternalInput")
    ra = nc.dram_tensor("ra", [N_ROWS, KR], F32, kind="ExternalInput")
    out = nc.dram_tensor("out", [5, 1], F32, kind="ExternalOutput")

    with TileContext(nc) as tc:
        with (
            tc.tile_pool(name="fa_p", bufs=6) as fa_p,
            tc.tile_pool(name="big_p", bufs=1) as big_p,
            tc.tile_pool(name="lhs_p", bufs=6) as lhs_p,
            tc.tile_pool(name="nrm_p", bufs=4) as nrm_p,
            tc.tile_pool(name="scl_p", bufs=4) as scl_p,
            tc.tile_pool(name="scr_p", bufs=4) as scr_p,
            tc.tile_pool(name="acc_p", bufs=1) as acc_p,
            tc.tile_pool(name="psum", bufs=6, space="PSUM") as psum_p,
        ):
            # --- constants / accumulators ---
            acc5 = acc_p.tile([P, 5], F32)
            ones = acc_p.tile([P, 1], F32)
            nc.vector.memset(ones[:], 1.0)
            nc.vector.memset(acc5[:], 0.0)
            # prime the ACT table set (sqrt_and_others: Square+Sqrt+Copy)
            # so the ~2.7us table load overlaps the first DMAs.
            warm = acc_p.tile([P, 1], F32)
            nc.scalar.activation(warm[:], ones[:], ACTF.Square)

            # --- R, all chunks in one [128, 16*128] tile; batch norms ---
            ra_all = big_p.tile([P, NCH * KR], F32R)
            nc.sync.dma_start(
                ra_all[:].rearrange("p (k j) -> p k j", j=KR),
                ra[:].rearrange("(k p) j -> p k j", p=P).bitcast(F32R),
            )
            rsq = big_p.tile([P, NCH * KR], F32)
            nc.vector.tensor_tensor(
                rsq[:], ra_all[:].bitcast(F32), ra_all[:].bitcast(F32),
                AluOpType.mult,
            )
            nr2 = nrm_p.tile([P, NCH], F32, tag="nr2")
            nc.vector.reduce_sum(
                nr2[:], rsq[:].rearrange("p (k j) -> p k j", j=KR), axis=AX.X
            )
            tr_all = nrm_p.tile([P, NCH], F32, tag="tr")
            br_all = nrm_p.tile([P, NCH], F32, tag="br")
            nc.vector.tensor_scalar_max(tr_all[:], nr2[:], EPS2)
            nc.vector.reciprocal(br_all[:], tr_all[:])

            # --- PSUM accumulators (5 banks) ---
            psA0 = psum_p.tile([P, 512], F32, tag="acc", name="psA0")
            psA1 = psum_p.tile([P, 512], F32, tag="acc", name="psA1")
            psX0 = psum_p.tile([P, 512], F32, tag="acc", name="psX0")
            psX1 = psum_p.tile([P, 512], F32, tag="acc", name="psX1")
            psB = psum_p.tile([P, KR], F32, tag="acc", name="psB")

            # --- main stream: per group of GRP chunks ---
            fa_sbs = [None] * NCH
            for g in range(NCH // GRP):
                nf2 = nrm_p.tile([P, GRP], F32, tag="nf2")
                for j in range(GRP):
                    ki = GRP * g + j
                    fa_sb = fa_p.tile([P, KF], F32R, tag="fa_sb")
                    nc.sync.dma_start(
                        fa_sb[:, 0:512],
                        fa[ki * P:(ki + 1) * P, 0:512].bitcast(F32R),
                    )
                    nc.sync.dma_start(
                        fa_sb[:, 512:KF],
                        fa[ki * P:(ki + 1) * P, 512:KF].bitcast(F32R),
                    )
                    fa_sbs[ki] = fa_sb
                    if ki in ACT_SQ:
                        scr = scr_p.tile([P, KF], F32, tag="scrA")
                        nc.scalar.activation(
                            scr[:], fa_sb[:].bitcast(F32), ACTF.Square,
                            accum_out=nf2[:, j:j + 1],
                        )
                    else:
                        scr = scr_p.tile([P, KF], F32, tag="scrV")
                        nc.vector.scalar_tensor_tensor(
                            scr[:], fa_sb[:].bitcast(F32), 1.0,
                            fa_sb[:].bitcast(F32),
                            AluOpType.mult, AluOpType.mult,
                            accum_out=nf2[:, j:j + 1],
                        )

                # batched scale math for this group
                tf = scl_p.tile([P, GRP], F32, tag="tf")
                af = scl_p.tile([P, GRP], F32, tag="af")
                uu = scl_p.tile([P, GRP], F32, tag="uu")
                ss = scl_p.tile([P, GRP], F32, tag="ss")
                gg = scl_p.tile([P, GRP], F32, tag="gg")
                nc.vector.tensor_scalar_max(tf[:], nf2[:], EPS2)
                nc.vector.reciprocal(af[:], tf[:])
                nc.vector.tensor_tensor(
                    uu[:], tf[:], tr_all[:, GRP * g:GRP * (g + 1)],
                    AluOpType.mult,
                )
                nc.scalar.activation(ss[:], uu[:], ACTF.Sqrt)
                nc.vector.reciprocal(gg[:], ss[:])

                # scaled stationary slices + the 5 matmuls per chunk
                for j in range(GRP):
                    ki = GRP * g + j
                    fa_sb = fa_sbs[ki]
                    la = lhs_p.tile([P, GF_W], F32R, tag="la")
                    nc.vector.tensor_scalar_mul(
                        la[:], fa_sb[:, 0:GF_W].bitcast(F32), af[:, j:j + 1]
                    )
                    lx = lhs_p.tile([P, X_W], F32R, tag="lx")
                    nc.vector.tensor_scalar_mul(
                        lx[:],
                        ra_all[:, ki * KR:ki * KR + X_W].bitcast(F32),
                        gg[:, j:j + 1],
                    )
                    lb = lhs_p.tile([P, X_W], F32R, tag="lb")
                    nc.vector.tensor_scalar_mul(
                        lb[:],
                        ra_all[:, ki * KR:ki * KR + X_W].bitcast(F32),
                        br_all[:, ki:ki + 1],
                    )
                    st = dict(start=(ki == 0), stop=(ki == NCH - 1))
                    nc.tensor.matmul(psA0[:], lhsT=la[:], rhs=fa_sb[:, 0:512], **st)
                    nc.tensor.matmul(psA1[:], lhsT=la[:], rhs=fa_sb[:, 512:KF], **st)
                    nc.tensor.matmul(psX0[0:X_W, :], lhsT=lx[:], rhs=fa_sb[:, 0:512], **st)
                    nc.tensor.matmul(psX1[0:X_W, :], lhsT=lx[:], rhs=fa_sb[:, 512:KF], **st)
                    nc.tensor.matmul(
                        psB[0:X_W, :], lhsT=lb[:],
                        rhs=ra_all[:, ki * KR:(ki + 1) * KR], **st
                    )

            # --- epilogue: Frobenius partials into acc5 cols ---
            for col, (ps, rows, w) in enumerate([
                (psA0, P, 512), (psA1, P, 512),
                (psX0, X_W, 512), (psX1, X_W, 512), (psB, X_W, KR),
            ]):
                scr = scr_p.tile([P, w], F32, tag="scrE", name=f"scrE{col}")
                nc.scalar.activation(
                    scr[0:rows, :], ps[0:rows, 0:w], ACTF.Square,
                    accum_out=acc5[0:rows, col:col + 1],
                )

            # partition-reduce via ones-matmul: out[5,1] = acc5^T @ ones
            psum_s = psum_p.tile([5, 1], F32, tag="acc", name="psS")
            nc.tensor.matmul(
                psum_s[:], lhsT=acc5[:], rhs=ones[:], start=True, stop=True
            )
            out_sb = acc_p.tile([5, 1], F32)
            nc.scalar.copy(out_sb[:], psum_s[:])
            nc.sync.dma_start(out[:], out_sb[:])

    nc.finalize()
    return nc


def kernel(reduced_embeddings: np.ndarray, full_embeddings: np.ndarray) -> np.ndarray:
    global _CACHED_NC, LAST_EXEC_NS
    from concourse.bass_utils import run_bass_kernel_spmd

    F = np.ascontiguousarray(full_embeddings, dtype=np.float32)
    R = np.ascontiguousarray(reduced_embeddings, dtype=np.float32)

    if _CACHED_NC is None:
        _CACHED_NC = _build()
    nc = _CACHED_NC

    # Shard: core c sees F rotated left by c*128 cols, R rotated by c*16.
    in_maps = []
    for c in range(8):
        fa = np.roll(F, -(c * GF_W), axis=1)
        ra = np.roll(R, -(c * X_W), axis=1)
        in_maps.append({"fa": np.ascontiguousarray(fa), "ra": np.ascontiguousarray(ra)})

    kw = {}
    if TRACE:
        kw = dict(trace=True, trace_cores=[0])
    res = run_bass_kernel_spmd(nc, in_maps, core_ids=list(range(8)), **kw)
    LAST_EXEC_NS = res.exec_time_ns

    # out rows: [Gf_lo, Gf_hi, X_lo, X_hi, Gr]; every core's piece is distinct.
    s_gf = sum(float(res.results[c]["out"][0, 0] + res.results[c]["out"][1, 0]) for c in range(8))
    s_x = sum(float(res.results[c]["out"][2, 0] + res.results[c]["out"][3, 0]) for c in range(8))
    s_gr = sum(float(res.results[c]["out"][4, 0]) for c in range(8))
    loss = (s_gf - 2.0 * s_x + s_gr) / (2.0 * M_PAIRS)
    return np.float32(loss)
